# revision 1
# baseline (speedup 1.0000x reference)
"""Trainium2 Bass kernel for nn_Attention_9242769622327.

Math: the reference computes
    qkv = x @ W1.T ; q,k,v = split(qkv)
    score = softmax(k^T v / 4, axis=-1)            # rows sum to 1
    attn  = softmax(einsum('bhnk,bhkc->bhnk', q/4, score), axis=-1)
          = softmax(q/4 * sum_c score)             # sum_c score == 1
          = softmax(q/4)                           # k/v are mathematically dead
    out   = attn @ W2.T
so only the q-projection (first E rows of W1), a per-head (64-wide) softmax,
and the output projection are needed.

Distribution: pure data-parallel over the 32768 = B*S rows; each of the 8
cores handles 4096 rows with the full (transposed, fp16) weights. No
collectives.  fp16 runs the PE at the same 1 cycle/row as bf16 but with a
10-bit mantissa (rel err ~4.5e-4 vs ~3.6e-3 for bf16).

On-chip layout is fully transposed (features on partitions, rows on the free
dim) so no on-chip transposes are needed anywhere:
    qT[n,m]  = sum_k W1qT[k,n] * xT[k,m]          (PE, fp16)
    u        = exp(qT/4)                          (ACT, PSUM->SBUF fp16)
    s[g,m]   = sum_{n in head g} u[n,m]           (PE matmul w/ 0/1 selector)
    rcp      = 1/s                                (DVE reciprocal_approx_fast;
                                                   no Ln -> one ACT table set)
    rb[n,m]  = rcp[head(n),m]                     (PE matmul w/ selector^T,
                                                   K padded to 128 so LDW overlaps)
    aT       = u * rb                             (DVE)
    outT[j,m]= sum_n W2T[n,j] * aT[n,m]           (PE, fp16)

Stripes are software-pipelined: stripe ms runs [64 mm1][8 rb(ms-1)][8 sel]
[64 mm2(ms-1)] as contiguous same-shape matmul blocks on the PE (back-to-back
216ns issue at N=512), with exp/reciprocal/normalize hidden underneath.
Measured: 280.8us on 8 cores, rel err 4.5e-4 (vs ~249us pure-matmul floor).
"""

import sys

sys.path.insert(0, "/opt/trn_rl_repo")

import numpy as np
import ml_dtypes

import concourse.bass as bass
import concourse.bacc as bacc
import concourse.tile as tile
from concourse import mybir
from concourse.bass_utils import run_bass_kernel_spmd

BF16 = mybir.dt.float16  # fp16: same PE rate as bf16, 10-bit mantissa
F32 = mybir.dt.float32
AF = mybir.ActivationFunctionType

N_CORES = 8
B, S, E = 4, 8192, 1024
HEADS, HEAD_DIM = 16, 64
M_TOTAL = B * S                # 32768
M_CORE = M_TOTAL // N_CORES    # 4096 rows per core
MS = 512                       # m-stripe width (moving free dim / PSUM bank)
N_STRIPES = M_CORE // MS       # 8
KC = E // 128                  # 8 contraction chunks
NC_ = E // 128                 # 8 feature chunks

_BF = np.float16


def build_nc() -> bass.Bass:
    nc = bacc.Bacc("TRN2", debug=False)

    xt = nc.dram_tensor("xt", [E, M_CORE], BF16, kind="ExternalInput")
    w1t = nc.dram_tensor("w1t", [E, E], BF16, kind="ExternalInput")
    w2t = nc.dram_tensor("w2t", [E, E], BF16, kind="ExternalInput")
    sel = nc.dram_tensor("sel", [128, NC_ * HEADS], BF16, kind="ExternalInput")
    selt = nc.dram_tensor("selt", [128, NC_ * 128], BF16, kind="ExternalInput")
    outT = nc.dram_tensor("outT", [E, M_CORE], BF16, kind="ExternalOutput")

    xt_v = xt[:, :].rearrange("(c p) m -> p c m", p=128)    # [128, 8, M_CORE]
    w1_v = w1t[:, :].rearrange("(c p) n -> p c n", p=128)   # [128, 8, 1024]
    w2_v = w2t[:, :].rearrange("(c p) j -> p c j", p=128)   # [128, 8, 1024]

    with tile.TileContext(nc) as tc:
        with (
            tc.tile_pool(name="weights", bufs=1) as wpool,
            tc.tile_pool(name="xt", bufs=N_STRIPES) as xpool,
            tc.tile_pool(name="u", bufs=16) as upool,
            tc.tile_pool(name="at", bufs=16) as apool,
            tc.tile_pool(name="small", bufs=3) as spool,
            tc.tile_pool(name="ostage", bufs=8) as opool,
            tc.tile_pool(name="ps_q", bufs=2, space="PSUM") as psq,
            tc.tile_pool(name="ps_s", bufs=2, space="PSUM") as pss,
            tc.tile_pool(name="ps_rb", bufs=2, space="PSUM") as psrb,
            tc.tile_pool(name="ps_o", bufs=2, space="PSUM") as pso,
        ):
            # Per-chunk weight tiles so the first matmuls only wait on the
            # chunks they read, not the whole 4MB of weights.  Load order:
            # w1 + sel (needed by stripe 0's mm1/sel), stripe-0 x chunks,
            # then w2 + selt (not needed until ~18us in).
            # Warm the PE's HAM clock gate with throwaway matmuls on memset
            # scratch while the first weight/x DMAs are in flight, so the
            # first real matmuls run at 2.4 GHz instead of 1.2.
            warm_sb = wpool.tile([128, MS], BF16, name="warm_sb")
            nc.gpsimd.memset(warm_sb[:], 0.0)
            warm_ps = psq.tile([128, MS], F32, tag="q", name="warm_ps")
            for _ in range(16):
                nc.tensor.matmul(
                    warm_ps[:], warm_sb[:, 0:128], warm_sb[:], start=True, stop=True
                )

            w1_k = []
            xt0 = []
            for kc in range(KC):
                t = wpool.tile([128, E], BF16, tag=f"w1_{kc}", name=f"w1k{kc}")
                nc.sync.dma_start(t[:], w1_v[:, kc, :])
                w1_k.append(t)
                tx = xpool.tile([128, MS], BF16, tag=f"xt_{kc}", name=f"xt0_{kc}")
                nc.sync.dma_start(tx[:], xt_v[:, kc, 0:MS])
                xt0.append(tx)
            sel_t = wpool.tile([128, NC_, HEADS], BF16, name="sel_t")
            nc.sync.dma_start(sel_t[:], sel[:, :].rearrange("p (c g) -> p c g", g=HEADS))

            w2_k = []
            for ci in range(NC_):
                t = wpool.tile([128, E], BF16, tag=f"w2_{ci}", name=f"w2k{ci}")
                nc.sync.dma_start(t[:], w2_v[:, ci, :])
                w2_k.append(t)
            selt_t = wpool.tile([128, NC_, 128], BF16, name="selt_t")
            nc.sync.dma_start(selt_t[:], selt[:, :].rearrange("p (c q) -> p c q", q=128))

            # Software pipeline over stripes: while stripe ms runs its
            # q-projection (mm1) + exp + head-sum on the PE, stripe ms-1's
            # normalization (rb broadcast matmul + DVE mul) and output
            # projection (mm2) are interleaved so the PE never waits on the
            # softmax chain.
            prev_u = None       # u tiles of stripe ms-1
            prev_rcp = None     # reciprocal head-sums of stripe ms-1 (bf16)
            prev_ms = -1

            def emit_norm(pu, prcp):
                """rb broadcast matmuls (PE, contiguous block, K padded to 128
                so LDWEIGHTS overlaps like the main GEMM blocks) + DVE muls."""
                ats = []
                for ci in range(NC_):
                    rb_ps = psrb.tile([128, MS], F32, tag="rb", name="rb_ps")
                    nc.tensor.matmul(
                        rb_ps[:], selt_t[:, ci, :], prcp[:], start=True, stop=True
                    )
                    at_t = apool.tile([128, MS], BF16, tag="at", name="at_t")
                    nc.vector.tensor_mul(at_t[:], pu[ci][:], rb_ps[:])
                    ats.append(at_t)
                return ats

            def emit_tail(at_list, ms):
                """Emit mm2 + store for a finished stripe (at tiles ready)."""
                for j in range(NC_):
                    o_ps = pso.tile([128, MS], F32, tag="o", name="o_ps")
                    for ci in range(NC_):
                        nc.tensor.matmul(
                            o_ps[:],
                            w2_k[ci][:, j * 128:(j + 1) * 128],
                            at_list[ci][:],
                            start=(ci == 0),
                            stop=(ci == NC_ - 1),
                        )
                    o_t = opool.tile([128, MS], BF16, tag="ost", name="o_t")
                    nc.scalar.copy(o_t[:], o_ps[:])
                    nc.sync.dma_start(
                        outT[j * 128:(j + 1) * 128, ms * MS:(ms + 1) * MS], o_t[:]
                    )

            for ms in range(N_STRIPES):
                if ms == 0:
                    xt_k = xt0
                else:
                    xt_k = []
                    for kc in range(KC):
                        t = xpool.tile(
                            [128, MS], BF16, tag=f"xt_{kc}", name=f"xt{ms}_{kc}"
                        )
                        nc.sync.dma_start(
                            t[:], xt_v[:, kc, ms * MS:(ms + 1) * MS]
                        )
                        xt_k.append(t)

                # ---- mm1: q-projection, contiguous 64-MM block on PE ----
                u_tiles = []
                q_list = []
                for ci in range(NC_):
                    q_ps = psq.tile([128, MS], F32, tag="q", name="q_ps")
                    for kc in range(KC):
                        nc.tensor.matmul(
                            q_ps[:],
                            w1_k[kc][:, ci * 128:(ci + 1) * 128],
                            xt_k[kc][:],
                            start=(kc == 0),
                            stop=(kc == KC - 1),
                        )
                    u_t = upool.tile([128, MS], BF16, tag="u", name="u_t")
                    nc.scalar.activation(u_t[:], q_ps[:], AF.Exp, scale=0.25)
                    u_tiles.append(u_t)

                # ---- stripe ms-1 normalization (hides exp latency) ----
                at_tiles = emit_norm(prev_u, prev_rcp) if prev_rcp is not None else None

                # ---- head sums (contiguous 8-MM block) + reciprocal ----
                s_ps = pss.tile([HEADS, MS], F32, tag="s", name="s_ps")
                for ci in range(NC_):
                    nc.tensor.matmul(
                        s_ps[:],
                        sel_t[:, ci, :],
                        u_tiles[ci][:],
                        start=(ci == 0),
                        stop=(ci == NC_ - 1),
                    )
                rcp32 = spool.tile([HEADS, MS], F32, tag="rcp32", name="rcp32")
                nc.vector.reciprocal_approx_fast(rcp32[:], s_ps[:])
                # rcp padded to 128 partitions (rows 16+ zeroed on the idle
                # GpSimd engine) so the rb matmul runs with K=128
                rcp_t = spool.tile([128, MS], BF16, tag="rcp", name="rcp_t")
                nc.gpsimd.memset(rcp_t[:], 0.0)
                nc.scalar.copy(rcp_t[0:HEADS, :], rcp32[:])

                # ---- stripe ms-1 output projection ----
                if at_tiles is not None:
                    emit_tail(at_tiles, prev_ms)
                prev_u, prev_rcp, prev_ms = u_tiles, rcp_t, ms

            # epilogue: last stripe's normalization + output projection
            at_tiles = emit_norm(prev_u, prev_rcp)
            emit_tail(at_tiles, prev_ms)
    nc.compile()
    return nc


_NC_CACHE = None
LAST_RESULT = None


def _ensure_ntff_hook():
    """bass_utils' axon trace path needs antenv.axon_hooks, which this
    container's antenv lacks. Provide it + register the ctypes NTFF hook."""
    import types

    try:
        from antenv.axon_hooks import get_axon_ntff_profile_hook  # noqa: F401
        return True
    except ImportError:
        pass
    try:
        import antenv
        from trn_agent_boot.trn_boot import _ntff_profile_via_ctypes

        m = types.ModuleType("antenv.axon_hooks")
        state = {"hook": None}
        m.set_axon_ntff_profile_hook = lambda h: state.__setitem__("hook", h)
        m.get_axon_ntff_profile_hook = lambda: state["hook"]
        sys.modules["antenv.axon_hooks"] = m
        antenv.axon_hooks = m
        m.set_axon_ntff_profile_hook(
            _ntff_profile_via_ctypes("/opt/axon/libaxon_pjrt.so")
        )
        return True
    except Exception as e:  # pragma: no cover
        print(f"ntff hook injection failed: {e}")
        return False


def _selectors():
    # head index of global feature n is n // 64; chunk ci covers n in
    # [128ci, 128ci+128) -> heads 2ci (partitions 0..63) and 2ci+1 (64..127)
    sel = np.zeros((128, NC_, HEADS), np.float32)
    selt = np.zeros((128, NC_, 128), np.float32)  # K padded to 128, rows 16+ zero
    for ci in range(NC_):
        sel[:64, ci, 2 * ci] = 1.0
        sel[64:, ci, 2 * ci + 1] = 1.0
        selt[2 * ci, ci, :64] = 1.0
        selt[2 * ci + 1, ci, 64:] = 1.0
    return (
        np.ascontiguousarray(sel.reshape(128, NC_ * HEADS)).astype(_BF),
        np.ascontiguousarray(selt.reshape(128, NC_ * 128)).astype(_BF),
    )


def kernel(x, W1, W2, heads, trace=False):
    global _NC_CACHE, LAST_RESULT
    x = np.asarray(x, dtype=np.float32)
    W1 = np.asarray(W1, dtype=np.float32)
    W2 = np.asarray(W2, dtype=np.float32)

    X = x.reshape(M_TOTAL, E)
    Xbf = X.astype(_BF)
    XbfT = Xbf.T  # [E, M_TOTAL] view
    w1t = np.ascontiguousarray(W1[:E, :].T).astype(_BF)   # [k, n] = W1q[n, k]
    w2t = np.ascontiguousarray(W2.T).astype(_BF)          # [n, j] = W2[j, n]
    sel, selt = _selectors()

    in_maps = []
    for c in range(N_CORES):
        xt_c = np.ascontiguousarray(XbfT[:, c * M_CORE:(c + 1) * M_CORE])
        in_maps.append(
            {"xt": xt_c, "w1t": w1t, "w2t": w2t, "sel": sel, "selt": selt}
        )

    if _NC_CACHE is None:
        _NC_CACHE = build_nc()

    if trace:
        trace = _ensure_ntff_hook()

    res = run_bass_kernel_spmd(_NC_CACHE, in_maps, list(range(N_CORES)), trace=trace)
    LAST_RESULT = res

    OT = np.concatenate(
        [np.asarray(res.results[c]["outT"]).astype(np.float32) for c in range(N_CORES)],
        axis=1,
    )
    return np.ascontiguousarray(OT.T).reshape(B, S, E)



# revision 3
# speedup vs baseline: 1.5575x; 1.5575x over previous
"""Trainium2 Bass kernel for nn_Attention_9242769622327.

Math: the reference computes
    qkv = x @ W1.T ; q,k,v = split(qkv)
    score = softmax(k^T v / 4, axis=-1)            # rows sum to 1
    attn  = softmax(einsum('bhnk,bhkc->bhnk', q/4, score), axis=-1)
          = softmax(q/4)                           # k/v are mathematically dead
    out   = attn @ W2.T
so only the q-projection (first E rows of W1), a per-head (64-wide) softmax,
and the output projection are needed.

Distribution: pure data-parallel over the 32768 = B*S rows; each of the 8
cores handles 4096 rows. No collectives.

Precision strategy (fp8 DoubleRow = 2 fp8 K-values per PE cell per cycle,
i.e. K=256 per matmul instruction -> half the instruction count):
  mm1 (q-projection) in fp8e4 DoubleRow: x ~ N(0,1) and 32*W1 ~ N(0,1)
      quantize to e4m3 with ~1.8% rel err each -> q abs err ~2.5e-2, and
      exp(q/4) divides it by 4 -> ~0.6% on attn. 32 instrs vs 64 fp16.
  mm2 (output projection) in fp8e4 DoubleRow via CENTERING: softmax over 64
      logits with sigma=0.25 gives at = 64*attn = 1 + delta, |delta|~0.25.
      out = (1/64)*(rowsum(W2T)[j] + delta @ W2T): the constant term is exact
      (per-partition bias on the output copy); only delta rides through fp8,
      so quantization error is ~4x smaller: ~0.45% per operand. 32 instrs.
  head-sum + rcp broadcast stay fp16 (8+8 instrs).

On-chip layout fully transposed (features on partitions, rows on free dim):
    qT[n,m]  = sum_k W1qT[k,n]*xT[k,m]     (PE, fp8 DoubleRow, 32 MM)
    u        = exp(qT/128)  [qT is 32x]    (ACT, PSUM->SBUF fp16)
    s[g,m]   = sum_{n in head g} u[n,m]    (PE fp16 w/ 0/1 selector, 8 MM)
    rcp      = 64/s                        (DVE reciprocal + ACT copy*64)
    rb[n,m]  = rcp[head(n),m]              (PE fp16 selector^T matmul, 8 MM)
    at       = u * rb                      (DVE, fp16)
    d8       = at - 1                      (DVE tensor_scalar, e4m3 out)
    oT[j,m]  = sum_n 32W2T[n,j]*d8[n,m]    (PE fp8 DoubleRow, 32 MM)
    outT     = oT/2048 + rowsumW2[j]/64    (ACT Identity w/ bias AP, fp16)

Per-stripe PE: 80 matmul instrs (~242ns each) vs baseline's 144.
"""

import sys

sys.path.insert(0, "/opt/trn_rl_repo")

import numpy as np
import ml_dtypes

import concourse.bass as bass
import concourse.bacc as bacc
import concourse.tile as tile
from concourse import mybir
from concourse.bass_utils import run_bass_kernel_spmd

FP16 = mybir.dt.float16
FP8 = mybir.dt.float8e4
F32 = mybir.dt.float32
AF = mybir.ActivationFunctionType
DR = mybir.MatmulPerfMode.DoubleRow

N_CORES = 8
B, S, E = 4, 8192, 1024
HEADS, HEAD_DIM = 16, 64
M_TOTAL = B * S                # 32768
M_CORE = M_TOTAL // N_CORES    # 4096 rows per core
MS = 512                       # m-stripe width (moving free dim / PSUM bank)
N_STRIPES = M_CORE // MS       # 8
KC2 = E // 256                 # 4 DoubleRow contraction chunks (K=256 each)
NC_ = E // 128                 # 8 feature chunks

_E4 = ml_dtypes.float8_e4m3
_F16 = np.float16


def build_nc() -> bass.Bass:
    nc = bacc.Bacc("TRN2", debug=False)

    xt = nc.dram_tensor("xt", [E, M_CORE], FP8, kind="ExternalInput")
    w1 = nc.dram_tensor("w1", [E, E], FP8, kind="ExternalInput")
    w2 = nc.dram_tensor("w2", [E, E], FP8, kind="ExternalInput")
    sel = nc.dram_tensor("sel", [128, NC_ * HEADS], FP16, kind="ExternalInput")
    selt = nc.dram_tensor("selt", [128, NC_ * 128], FP16, kind="ExternalInput")
    bias = nc.dram_tensor("bias", [128, NC_], F32, kind="ExternalInput")
    outT = nc.dram_tensor("outT", [E, M_CORE], FP16, kind="ExternalOutput")

    # row k of xt/w1 maps to (c, i, p): k = c*256 + i*128 + p  (DoubleRow pair
    # slot i); same for w2 rows n = t*256 + i*128 + p.
    xt_v = xt[:, :].rearrange("(c i p) m -> p c i m", p=128, i=2)
    w1_v = w1[:, :].rearrange("(c i p) n -> p c i n", p=128, i=2)
    w2_v = w2[:, :].rearrange("(t i p) j -> p t i j", p=128, i=2)

    with tile.TileContext(nc) as tc:
        with (
            tc.tile_pool(name="weights", bufs=1) as wpool,
            tc.tile_pool(name="xt", bufs=N_STRIPES) as xpool,
            tc.tile_pool(name="u", bufs=16) as upool,
            tc.tile_pool(name="at", bufs=16) as apool,
            tc.tile_pool(name="d8", bufs=8) as dpool,
            tc.tile_pool(name="small", bufs=3) as spool,
            tc.tile_pool(name="ostage", bufs=8) as opool,
            tc.tile_pool(name="ps_q", bufs=2, space="PSUM") as psq,
            tc.tile_pool(name="ps_s", bufs=2, space="PSUM") as pss,
            tc.tile_pool(name="ps_rb", bufs=2, space="PSUM") as psrb,
            tc.tile_pool(name="ps_o", bufs=2, space="PSUM") as pso,
        ):
            # Warm the PE's HAM clock gate with throwaway matmuls on memset
            # scratch while the first weight/x DMAs are in flight.
            warm_sb = wpool.tile([128, MS], FP16, name="warm_sb")
            nc.gpsimd.memset(warm_sb[:], 0.0)
            warm_ps = psq.tile([128, MS], F32, tag="q", name="warm_ps")
            for _ in range(16):
                nc.tensor.matmul(
                    warm_ps[:], warm_sb[:, 0:128], warm_sb[:], start=True, stop=True
                )

            # Load order: w1 + sel chunks (needed by stripe 0's mm1/sum),
            # stripe-0 x chunks, then w2/selt/bias (needed ~15us in).
            w1_c = []
            xt0 = []
            for c in range(KC2):
                t = wpool.tile([128, 2, E], FP8, tag=f"w1_{c}", name=f"w1c{c}")
                nc.sync.dma_start(t[:], w1_v[:, c, :, :])
                w1_c.append(t)
                tx = xpool.tile([128, 2, MS], FP8, tag=f"xt_{c}", name=f"xt0_{c}")
                nc.sync.dma_start(tx[:], xt_v[:, c, :, 0:MS])
                xt0.append(tx)
            sel_t = wpool.tile([128, NC_, HEADS], FP16, name="sel_t")
            nc.sync.dma_start(sel_t[:], sel[:, :].rearrange("p (c g) -> p c g", g=HEADS))

            w2_c = []
            for tt in range(KC2):
                t = wpool.tile([128, 2, E], FP8, tag=f"w2_{tt}", name=f"w2c{tt}")
                nc.sync.dma_start(t[:], w2_v[:, tt, :, :])
                w2_c.append(t)
            selt_t = wpool.tile([128, NC_, 128], FP16, name="selt_t")
            nc.sync.dma_start(selt_t[:], selt[:, :].rearrange("p (c q) -> p c q", q=128))
            bias_t = wpool.tile([128, NC_], F32, name="bias_t")
            nc.sync.dma_start(bias_t[:], bias[:, :])

            # Software pipeline over stripes: while stripe ms runs mm1 + exp +
            # head-sum, stripe ms-1 runs normalization (rb + DVE mul + delta
            # cast) and the output projection, so the PE never waits on the
            # softmax chain.
            prev_u = None
            prev_rcp = None
            prev_ms = -1

            def emit_norm(pu, prcp):
                """rb broadcast matmuls + DVE mul (at=u*rb) + delta = at-1
                cast to e4m3 pair tiles for the DoubleRow mm2."""
                dts = []
                for tt in range(KC2):
                    dt_t = dpool.tile([128, 2, MS], FP8, tag=f"d8_{tt}", name=f"d8_{tt}")
                    dts.append(dt_t)
                for ci in range(NC_):
                    rb_ps = psrb.tile([128, MS], F32, tag="rb", name="rb_ps")
                    nc.tensor.matmul(
                        rb_ps[:], selt_t[:, ci, :], prcp[:], start=True, stop=True
                    )
                    at_t = apool.tile([128, MS], FP16, tag="at", name="at_t")
                    nc.vector.tensor_mul(at_t[:], pu[ci][:], rb_ps[:])
                    nc.vector.tensor_scalar_sub(
                        dts[ci // 2][:, ci % 2, :], at_t[:], 1.0
                    )
                return dts

            def emit_tail(dts, ms):
                """mm2 (DoubleRow over delta pairs) + biased out-copy + store."""
                for j in range(NC_):
                    o_ps = pso.tile([128, MS], F32, tag="o", name="o_ps")
                    for tt in range(KC2):
                        nc.tensor.matmul(
                            o_ps[:],
                            w2_c[tt][:, :, j * 128:(j + 1) * 128],
                            dts[tt][:],
                            start=(tt == 0),
                            stop=(tt == KC2 - 1),
                            perf_mode=DR,
                        )
                    o_t = opool.tile([128, MS], FP16, tag="ost", name="o_t")
                    nc.scalar.activation(
                        o_t[:], o_ps[:], AF.Identity,
                        bias=bias_t[:, j:j + 1], scale=1.0 / 2048.0,
                    )
                    nc.sync.dma_start(
                        outT[j * 128:(j + 1) * 128, ms * MS:(ms + 1) * MS], o_t[:]
                    )

            for ms in range(N_STRIPES):
                if ms == 0:
                    xt_k = xt0
                else:
                    xt_k = []
                    for c in range(KC2):
                        t = xpool.tile(
                            [128, 2, MS], FP8, tag=f"xt_{c}", name=f"xt{ms}_{c}"
                        )
                        nc.sync.dma_start(
                            t[:], xt_v[:, c, :, ms * MS:(ms + 1) * MS]
                        )
                        xt_k.append(t)

                # ---- mm1: q-projection, 32 contiguous DoubleRow MMs ----
                u_tiles = []
                for ci in range(NC_):
                    q_ps = psq.tile([128, MS], F32, tag="q", name="q_ps")
                    for c in range(KC2):
                        nc.tensor.matmul(
                            q_ps[:],
                            w1_c[c][:, :, ci * 128:(ci + 1) * 128],
                            xt_k[c][:],
                            start=(c == 0),
                            stop=(c == KC2 - 1),
                            perf_mode=DR,
                        )
                    u_t = upool.tile([128, MS], FP16, tag="u", name="u_t")
                    nc.scalar.activation(u_t[:], q_ps[:], AF.Exp, scale=1.0 / 128.0)
                    u_tiles.append(u_t)

                # ---- stripe ms-1 normalization (hides exp latency) ----
                d_tiles = emit_norm(prev_u, prev_rcp) if prev_rcp is not None else None

                # ---- head sums (contiguous 8-MM block) + reciprocal ----
                s_ps = pss.tile([HEADS, MS], F32, tag="s", name="s_ps")
                for ci in range(NC_):
                    nc.tensor.matmul(
                        s_ps[:],
                        sel_t[:, ci, :],
                        u_tiles[ci][:],
                        start=(ci == 0),
                        stop=(ci == NC_ - 1),
                    )
                rcp32 = spool.tile([HEADS, MS], F32, tag="rcp32", name="rcp32")
                nc.vector.reciprocal_approx_fast(rcp32[:], s_ps[:])
                # rcp padded to 128 partitions (rows 16+ zeroed on the idle
                # GpSimd engine) so the rb matmul runs with K=128
                rcp_t = spool.tile([128, MS], FP16, tag="rcp", name="rcp_t")
                nc.gpsimd.memset(rcp_t[:], 0.0)
                nc.scalar.activation(
                    rcp_t[0:HEADS, :], rcp32[:], AF.Copy, scale=64.0
                )

                # ---- stripe ms-1 output projection ----
                if d_tiles is not None:
                    emit_tail(d_tiles, prev_ms)
                prev_u, prev_rcp, prev_ms = u_tiles, rcp_t, ms

            # epilogue: last stripe's normalization + output projection
            d_tiles = emit_norm(prev_u, prev_rcp)
            emit_tail(d_tiles, prev_ms)
    nc.compile()
    return nc


_NC_CACHE = None
LAST_RESULT = None


def _ensure_ntff_hook():
    """bass_utils' axon trace path needs antenv.axon_hooks, which this
    container's antenv lacks. Provide it + register the ctypes NTFF hook."""
    import types

    try:
        from antenv.axon_hooks import get_axon_ntff_profile_hook  # noqa: F401
        return True
    except ImportError:
        pass
    try:
        import antenv
        from trn_agent_boot.trn_boot import _ntff_profile_via_ctypes

        m = types.ModuleType("antenv.axon_hooks")
        state = {"hook": None}
        m.set_axon_ntff_profile_hook = lambda h: state.__setitem__("hook", h)
        m.get_axon_ntff_profile_hook = lambda: state["hook"]
        sys.modules["antenv.axon_hooks"] = m
        antenv.axon_hooks = m
        m.set_axon_ntff_profile_hook(
            _ntff_profile_via_ctypes("/opt/axon/libaxon_pjrt.so")
        )
        return True
    except Exception as e:  # pragma: no cover
        print(f"ntff hook injection failed: {e}")
        return False


def _selectors():
    # head index of global feature n is n // 64; chunk ci covers n in
    # [128ci, 128ci+128) -> heads 2ci (partitions 0..63) and 2ci+1 (64..127)
    sel = np.zeros((128, NC_, HEADS), np.float32)
    selt = np.zeros((128, NC_, 128), np.float32)  # K padded to 128, rows 16+ zero
    for ci in range(NC_):
        sel[:64, ci, 2 * ci] = 1.0
        sel[64:, ci, 2 * ci + 1] = 1.0
        selt[2 * ci, ci, :64] = 1.0
        selt[2 * ci + 1, ci, 64:] = 1.0
    return (
        np.ascontiguousarray(sel.reshape(128, NC_ * HEADS)).astype(_F16),
        np.ascontiguousarray(selt.reshape(128, NC_ * 128)).astype(_F16),
    )


def kernel(x, W1, W2, heads, trace=False):
    global _NC_CACHE, LAST_RESULT
    x = np.asarray(x, dtype=np.float32)
    W1 = np.asarray(W1, dtype=np.float32)
    W2 = np.asarray(W2, dtype=np.float32)

    X = x.reshape(M_TOTAL, E)
    Xq = X.astype(_E4)
    XqT = Xq.T  # [E, M_TOTAL] view
    w1q = np.ascontiguousarray(32.0 * W1[:E, :].T).astype(_E4)   # [k, n]
    w2q = np.ascontiguousarray(32.0 * W2.T).astype(_E4)          # [n, j]
    # bias[p, j] = rowsum(W2T)[j*128+p] / 64
    bias = np.ascontiguousarray(
        (W2.sum(axis=1) / 64.0).reshape(NC_, 128).T
    ).astype(np.float32)
    sel, selt = _selectors()

    in_maps = []
    for c in range(N_CORES):
        xt_c = np.ascontiguousarray(XqT[:, c * M_CORE:(c + 1) * M_CORE])
        in_maps.append(
            {"xt": xt_c, "w1": w1q, "w2": w2q, "sel": sel, "selt": selt,
             "bias": bias}
        )

    if _NC_CACHE is None:
        _NC_CACHE = build_nc()

    if trace:
        trace = _ensure_ntff_hook()

    res = run_bass_kernel_spmd(_NC_CACHE, in_maps, list(range(N_CORES)), trace=trace)
    LAST_RESULT = res

    OT = np.concatenate(
        [np.asarray(res.results[c]["outT"]).astype(np.float32) for c in range(N_CORES)],
        axis=1,
    )
    return np.ascontiguousarray(OT.T).reshape(B, S, E)


# revision 5
# speedup vs baseline: 1.5807x; 1.0149x over previous
"""Trainium2 Bass kernel for nn_Attention_9242769622327.

Math: the reference computes
    qkv = x @ W1.T ; q,k,v = split(qkv)
    score = softmax(k^T v / 4, axis=-1)            # rows sum to 1
    attn  = softmax(einsum('bhnk,bhkc->bhnk', q/4, score), axis=-1)
          = softmax(q/4)                           # k/v are mathematically dead
    out   = attn @ W2.T
so only the q-projection (first E rows of W1), a per-head (64-wide) softmax,
and the output projection are needed.

Distribution: pure data-parallel over the 32768 = B*S rows; each of the 8
cores handles 4096 rows. No collectives.

Precision strategy (fp8 DoubleRow = 2 fp8 K-values per PE cell per cycle,
i.e. K=256 per matmul instruction -> half the instruction count):
  mm1 (q-projection) in fp8e4 DoubleRow: x ~ N(0,1) and 32*W1 ~ N(0,1)
      quantize to e4m3 with ~1.8% rel err each -> q abs err ~2.5e-2, and
      exp(q/4) divides it by 4 -> ~0.6% on attn. 32 instrs vs 64 fp16.
  mm2 (output projection) in fp8e4 DoubleRow via CENTERING: softmax over 64
      logits with sigma=0.25 gives at = 64*attn = 1 + delta, |delta|~0.25.
      out = (1/64)*(rowsum(W2T)[j] + delta @ W2T): the constant term is exact
      (per-partition bias on the output copy); only delta rides through fp8,
      so quantization error is ~4x smaller: ~0.45% per operand. 32 instrs.
  head-sum + rcp broadcast stay fp16 (8+8 instrs).

On-chip layout fully transposed (features on partitions, rows on free dim):
    qT[n,m]  = sum_k W1qT[k,n]*xT[k,m]     (PE, fp8 DoubleRow, 32 MM)
    u        = exp(qT/128)  [qT is 32x]    (ACT, PSUM->SBUF fp16)
    s[g,m]   = sum_{n in head g} u[n,m]    (PE fp16 w/ 0/1 selector, 8 MM)
    rcp      = 64/s                        (DVE reciprocal + ACT copy*64)
    rb[n,m]  = rcp[head(n),m]              (PE fp16 selector^T matmul, 8 MM)
    at       = u * rb                      (DVE, fp16)
    d8       = at - 1                      (DVE tensor_scalar, e4m3 out)
    oT[j,m]  = sum_n 32W2T[n,j]*d8[n,m]    (PE fp8 DoubleRow, 32 MM)
    outT     = oT/2048 + rowsumW2[j]/64    (ACT Identity w/ bias AP, fp16)

Per-stripe PE: 80 matmul instrs (~242ns each) vs baseline's 144.
"""

import sys

sys.path.insert(0, "/opt/trn_rl_repo")

import numpy as np
import ml_dtypes

import concourse.bass as bass
import concourse.bacc as bacc
import concourse.tile as tile
from concourse import mybir
from concourse.bass_utils import run_bass_kernel_spmd

FP16 = mybir.dt.float16
FP8 = mybir.dt.float8e4
F32 = mybir.dt.float32
AF = mybir.ActivationFunctionType
DR = mybir.MatmulPerfMode.DoubleRow

N_CORES = 8
B, S, E = 4, 8192, 1024
HEADS, HEAD_DIM = 16, 64
M_TOTAL = B * S                # 32768
M_CORE = M_TOTAL // N_CORES    # 4096 rows per core
MS = 512                       # m-stripe width (moving free dim / PSUM bank)
N_STRIPES = M_CORE // MS       # 8
KC2 = E // 256                 # 4 DoubleRow contraction chunks (K=256 each)
NC_ = E // 128                 # 8 feature chunks

_E4 = ml_dtypes.float8_e4m3
_F16 = np.float16


def build_nc() -> bass.Bass:
    nc = bacc.Bacc("TRN2", debug=False)

    xt = nc.dram_tensor("xt", [E, M_CORE], FP8, kind="ExternalInput")
    w1 = nc.dram_tensor("w1", [E, E], FP8, kind="ExternalInput")
    w2 = nc.dram_tensor("w2", [E, E], FP8, kind="ExternalInput")
    sel = nc.dram_tensor("sel", [128, NC_ * HEADS], FP16, kind="ExternalInput")
    selt = nc.dram_tensor("selt", [128, NC_ * 128], FP16, kind="ExternalInput")
    bias = nc.dram_tensor("bias", [128, NC_], F32, kind="ExternalInput")
    outT = nc.dram_tensor("outT", [E, M_CORE], FP16, kind="ExternalOutput")

    # row k of xt/w1 maps to (c, i, p): k = c*256 + i*128 + p  (DoubleRow pair
    # slot i); same for w2 rows n = t*256 + i*128 + p.
    xt_v = xt[:, :].rearrange("(c i p) m -> p c i m", p=128, i=2)
    w1_v = w1[:, :].rearrange("(c i p) n -> p c i n", p=128, i=2)
    w2_v = w2[:, :].rearrange("(t i p) j -> p t i j", p=128, i=2)

    with tile.TileContext(nc) as tc:
        with (
            tc.tile_pool(name="weights", bufs=1) as wpool,
            tc.tile_pool(name="xt", bufs=N_STRIPES) as xpool,
            tc.tile_pool(name="u", bufs=16) as upool,
            tc.tile_pool(name="at", bufs=16) as apool,
            tc.tile_pool(name="d8", bufs=8) as dpool,
            tc.tile_pool(name="small", bufs=3) as spool,
            tc.tile_pool(name="ostage", bufs=8) as opool,
            tc.tile_pool(name="ps_q", bufs=2, space="PSUM") as psq,
            tc.tile_pool(name="ps_s", bufs=2, space="PSUM") as pss,
            tc.tile_pool(name="ps_rb", bufs=2, space="PSUM") as psrb,
            tc.tile_pool(name="ps_o", bufs=2, space="PSUM") as pso,
        ):
            # Warm the PE's HAM clock gate with throwaway matmuls on memset
            # scratch while the first weight/x DMAs are in flight.
            warm_sb = wpool.tile([128, MS], FP16, name="warm_sb")
            nc.gpsimd.memset(warm_sb[:], 0.0)
            warm_ps = psq.tile([128, MS], F32, tag="q", name="warm_ps")
            for _ in range(16):
                nc.tensor.matmul(
                    warm_ps[:], warm_sb[:, 0:128], warm_sb[:], start=True, stop=True
                )

            # Load order: w1 + sel chunks (needed by stripe 0's mm1/sum),
            # stripe-0 x chunks, then w2/selt/bias (needed ~15us in).
            w1_c = []
            xt0 = []
            for c in range(KC2):
                t = wpool.tile([128, 2, E], FP8, tag=f"w1_{c}", name=f"w1c{c}")
                nc.sync.dma_start(t[:], w1_v[:, c, :, :])
                w1_c.append(t)
                tx = xpool.tile([128, 2, MS], FP8, tag=f"xt_{c}", name=f"xt0_{c}")
                nc.sync.dma_start(tx[:], xt_v[:, c, :, 0:MS])
                xt0.append(tx)
            sel_t = wpool.tile([128, NC_, HEADS], FP16, name="sel_t")
            nc.sync.dma_start(sel_t[:], sel[:, :].rearrange("p (c g) -> p c g", g=HEADS))

            w2_c = []
            for tt in range(KC2):
                t = wpool.tile([128, 2, E], FP8, tag=f"w2_{tt}", name=f"w2c{tt}")
                nc.sync.dma_start(t[:], w2_v[:, tt, :, :])
                w2_c.append(t)
            selt_t = wpool.tile([128, NC_, 128], FP16, name="selt_t")
            nc.sync.dma_start(selt_t[:], selt[:, :].rearrange("p (c q) -> p c q", q=128))
            bias_t = wpool.tile([128, NC_], F32, name="bias_t")
            nc.sync.dma_start(bias_t[:], bias[:, :])

            # Software pipeline over stripes: while stripe ms runs mm1 + exp +
            # head-sum, stripe ms-1 runs normalization (rb + DVE mul + delta
            # cast) and the output projection, so the PE never waits on the
            # softmax chain.
            prev_u = None
            prev_rcp = None
            prev_ms = -1

            def emit_norm(pu, prcp):
                """rb broadcast matmuls + DVE mul (at=u*rb) + delta = at-1
                cast to e4m3 pair tiles for the DoubleRow mm2."""
                dts = []
                for tt in range(KC2):
                    dt_t = dpool.tile([128, 2, MS], FP8, tag=f"d8_{tt}", name=f"d8_{tt}")
                    dts.append(dt_t)
                for ci in range(NC_):
                    rb_ps = psrb.tile([128, MS], F32, tag="rb", name="rb_ps")
                    nc.tensor.matmul(
                        rb_ps[:], selt_t[:, ci, :], prcp[:], start=True, stop=True
                    )
                    at_t = apool.tile([128, MS], FP16, tag="at", name="at_t")
                    nc.vector.tensor_mul(at_t[:], pu[ci][:], rb_ps[:])
                    nc.vector.tensor_scalar_sub(
                        dts[ci // 2][:, ci % 2, :], at_t[:], 1.0
                    )
                return dts

            def emit_tail(dts, ms):
                """mm2 (DoubleRow over delta pairs) + biased out-copy + store."""
                for j in range(NC_):
                    o_ps = pso.tile([128, MS], F32, tag="o", name="o_ps")
                    for tt in range(KC2):
                        nc.tensor.matmul(
                            o_ps[:],
                            w2_c[tt][:, :, j * 128:(j + 1) * 128],
                            dts[tt][:],
                            start=(tt == 0),
                            stop=(tt == KC2 - 1),
                            perf_mode=DR,
                        )
                    o_t = opool.tile([128, MS], FP16, tag="ost", name="o_t")
                    nc.scalar.activation(
                        o_t[:], o_ps[:], AF.Identity,
                        bias=bias_t[:, j:j + 1], scale=1.0 / 2048.0,
                    )
                    nc.sync.dma_start(
                        outT[j * 128:(j + 1) * 128, ms * MS:(ms + 1) * MS], o_t[:]
                    )

            for ms in range(N_STRIPES):
                if ms == 0:
                    xt_k = xt0
                else:
                    xt_k = []
                    for c in range(KC2):
                        t = xpool.tile(
                            [128, 2, MS], FP8, tag=f"xt_{c}", name=f"xt{ms}_{c}"
                        )
                        nc.sync.dma_start(
                            t[:], xt_v[:, c, :, ms * MS:(ms + 1) * MS]
                        )
                        xt_k.append(t)

                # ---- stripe ms-1 normalization FIRST: its rb matmuls enter
                # the PE queue before mm1, so the DVE mul+sub chain for the
                # previous stripe's delta tiles runs under mm1+sum (~11.6us)
                # and mm2 never stalls on it.
                d_tiles = emit_norm(prev_u, prev_rcp) if prev_rcp is not None else None

                # ---- mm1: q-projection, 32 contiguous DoubleRow MMs ----
                u_tiles = []
                for ci in range(NC_):
                    q_ps = psq.tile([128, MS], F32, tag="q", name="q_ps")
                    for c in range(KC2):
                        nc.tensor.matmul(
                            q_ps[:],
                            w1_c[c][:, :, ci * 128:(ci + 1) * 128],
                            xt_k[c][:],
                            start=(c == 0),
                            stop=(c == KC2 - 1),
                            perf_mode=DR,
                        )
                    u_t = upool.tile([128, MS], FP16, tag="u", name="u_t")
                    nc.scalar.activation(u_t[:], q_ps[:], AF.Exp, scale=1.0 / 128.0)
                    u_tiles.append(u_t)

                # ---- head sums (contiguous 8-MM block) + reciprocal ----
                s_ps = pss.tile([HEADS, MS], F32, tag="s", name="s_ps")
                for ci in range(NC_):
                    nc.tensor.matmul(
                        s_ps[:],
                        sel_t[:, ci, :],
                        u_tiles[ci][:],
                        start=(ci == 0),
                        stop=(ci == NC_ - 1),
                    )
                rcp32 = spool.tile([HEADS, MS], F32, tag="rcp32", name="rcp32")
                nc.vector.reciprocal_approx_fast(rcp32[:], s_ps[:])
                # rcp padded to 128 partitions (rows 16+ zeroed on the idle
                # GpSimd engine) so the rb matmul runs with K=128
                rcp_t = spool.tile([128, MS], FP16, tag="rcp", name="rcp_t")
                nc.gpsimd.memset(rcp_t[:], 0.0)
                nc.scalar.activation(
                    rcp_t[0:HEADS, :], rcp32[:], AF.Copy, scale=64.0
                )

                # ---- stripe ms-1 output projection ----
                if d_tiles is not None:
                    emit_tail(d_tiles, prev_ms)
                prev_u, prev_rcp, prev_ms = u_tiles, rcp_t, ms

            # epilogue: last stripe's normalization + output projection
            d_tiles = emit_norm(prev_u, prev_rcp)
            emit_tail(d_tiles, prev_ms)
    nc.compile()
    return nc


_NC_CACHE = None
LAST_RESULT = None


def _ensure_ntff_hook():
    """bass_utils' axon trace path needs antenv.axon_hooks, which this
    container's antenv lacks. Provide it + register the ctypes NTFF hook."""
    import types

    try:
        from antenv.axon_hooks import get_axon_ntff_profile_hook  # noqa: F401
        return True
    except ImportError:
        pass
    try:
        import antenv
        from trn_agent_boot.trn_boot import _ntff_profile_via_ctypes

        m = types.ModuleType("antenv.axon_hooks")
        state = {"hook": None}
        m.set_axon_ntff_profile_hook = lambda h: state.__setitem__("hook", h)
        m.get_axon_ntff_profile_hook = lambda: state["hook"]
        sys.modules["antenv.axon_hooks"] = m
        antenv.axon_hooks = m
        m.set_axon_ntff_profile_hook(
            _ntff_profile_via_ctypes("/opt/axon/libaxon_pjrt.so")
        )
        return True
    except Exception as e:  # pragma: no cover
        print(f"ntff hook injection failed: {e}")
        return False


def _selectors():
    # head index of global feature n is n // 64; chunk ci covers n in
    # [128ci, 128ci+128) -> heads 2ci (partitions 0..63) and 2ci+1 (64..127)
    sel = np.zeros((128, NC_, HEADS), np.float32)
    selt = np.zeros((128, NC_, 128), np.float32)  # K padded to 128, rows 16+ zero
    for ci in range(NC_):
        sel[:64, ci, 2 * ci] = 1.0
        sel[64:, ci, 2 * ci + 1] = 1.0
        selt[2 * ci, ci, :64] = 1.0
        selt[2 * ci + 1, ci, 64:] = 1.0
    return (
        np.ascontiguousarray(sel.reshape(128, NC_ * HEADS)).astype(_F16),
        np.ascontiguousarray(selt.reshape(128, NC_ * 128)).astype(_F16),
    )


def kernel(x, W1, W2, heads, trace=False):
    global _NC_CACHE, LAST_RESULT
    x = np.asarray(x, dtype=np.float32)
    W1 = np.asarray(W1, dtype=np.float32)
    W2 = np.asarray(W2, dtype=np.float32)

    X = x.reshape(M_TOTAL, E)
    Xq = X.astype(_E4)
    XqT = Xq.T  # [E, M_TOTAL] view
    w1q = np.ascontiguousarray(32.0 * W1[:E, :].T).astype(_E4)   # [k, n]
    w2q = np.ascontiguousarray(32.0 * W2.T).astype(_E4)          # [n, j]
    # bias[p, j] = rowsum(W2T)[j*128+p] / 64
    bias = np.ascontiguousarray(
        (W2.sum(axis=1) / 64.0).reshape(NC_, 128).T
    ).astype(np.float32)
    sel, selt = _selectors()

    in_maps = []
    for c in range(N_CORES):
        xt_c = np.ascontiguousarray(XqT[:, c * M_CORE:(c + 1) * M_CORE])
        in_maps.append(
            {"xt": xt_c, "w1": w1q, "w2": w2q, "sel": sel, "selt": selt,
             "bias": bias}
        )

    if _NC_CACHE is None:
        _NC_CACHE = build_nc()

    if trace:
        trace = _ensure_ntff_hook()

    res = run_bass_kernel_spmd(_NC_CACHE, in_maps, list(range(N_CORES)), trace=trace)
    LAST_RESULT = res

    OT = np.concatenate(
        [np.asarray(res.results[c]["outT"]).astype(np.float32) for c in range(N_CORES)],
        axis=1,
    )
    return np.ascontiguousarray(OT.T).reshape(B, S, E)
